# revision 2
# baseline (speedup 1.0000x reference)
"""Multi-head causal self-attention (B=4, T=2048, C=768, H=12) on 8 trn2 cores.

Sharding: core c handles batch b = c//2 and head-group hg = c%2 (6 heads).
No cross-core collectives; host sums the two output-projection partials per
batch and adds biases (k-bias cancels in softmax; v-bias is folded into b_o
host-side).

Cost-model-guided design:
- Host pre-transposes, pre-permutes and pre-casts all operands. QKV
  projections, scores and PV run as fp8e4 DoubleRow matmuls (2 contraction
  rows/partition), except a bf16 "early region" (queries 0..511) that kills
  the dominant quantization error of tiny-softmax-support early tokens.
- exp splits across ACT (native exp -> fp8/bf16 out) and DVE (Schraudolph:
  one tensor_scalar mult+add writing uint8 bit patterns that ARE fp8e4m3),
  routed by a build-time greedy balancer. Verified bit-exact on HW (rint).
- The PV stationary operand carries 64 appended ones-columns, so PSUM rows
  64:128 hold the softmax denominator replicated per-partition:
  normalization is a DVE reciprocal + lane-aligned multiply, no broadcast.
- Causal masking: one [128,128] gpsimd affine_select per diagonal block on
  the int8 view of fp8 att; fully-masked regions are skipped by splitting
  PV matmuls at 128-column boundaries.
- k/q fp8 pair layout (DoubleRow contraction) is produced by SBUF->SBUF
  partition-shuffle DMAs issued from the otherwise-idle SP sequencer.
"""

import math
import os

import numpy as np
import ml_dtypes

from concourse import bacc
import concourse.mybir as mybir
import concourse.tile as tile
from concourse import bass_utils
from concourse.bass import ts

F32 = mybir.dt.float32
BF16 = mybir.dt.bfloat16
F8 = mybir.dt.float8e4
I8 = mybir.dt.int8
U8 = mybir.dt.uint8
DR = mybir.MatmulPerfMode.DoubleRow
EXPF = mybir.ActivationFunctionType.Exp
IDF = mybir.ActivationFunctionType.Identity
NPF8 = ml_dtypes.float8_e4m3
NPBF16 = ml_dtypes.bfloat16

P = 128
T = 2048
C = 768
HL = 6            # heads per core
HD = 64
J = HL * HD       # 384 local y-feature dims
WS = 256.0        # host-side fp8 weight pre-scale (power of two)
SCALE = 0.125     # 1/sqrt(64)
A8 = SCALE * 8.0 / math.log(2.0)   # Schraudolph fp8e4m3 slope (= 1/ln2)
B8 = 56.0                          # fp8e4m3 exponent bias * 8


class Router:
    """Build-time elementwise work router between ACT and DVE.

    pick() alternates strictly (temporal interleaving matters more than
    cumulative balance: greedy-balance creates same-engine convoys that
    stall the scores->exp->PV pipeline); the alternation PATTERN is biased
    toward ACT, which is cheaper per column."""

    PATTERN = ["act", "dve", "act", "dve", "act"]

    def __init__(self):
        self.busy = {"act": 0.0, "dve": 0.0}
        self.i = 0

    def _cost(self, eng, cols):
        if eng == "act":
            return cols * 0.833 + 185 + 57
        return cols * 1.042 + 125 + 70

    def pick(self, cols):
        eng = self.PATTERN[self.i % len(self.PATTERN)]
        self.i += 1
        self.busy[eng] += self._cost(eng, cols)
        return eng

    def charge(self, eng, cols):
        self.busy[eng] += self._cost(eng, cols)


def _build_bass():
    nc = bacc.Bacc("TRN2", target_bir_lowering=False, debug=False)
    xt8_d = nc.dram_tensor("xt8", [P, 3, 2, T], F8, kind="ExternalInput").ap()
    xtb_d = nc.dram_tensor("xtb", [P, 6, 512], BF16, kind="ExternalInput").ap()
    wt8_d = nc.dram_tensor("wt8", [P, 3, 2, 768], F8, kind="ExternalInput").ap()
    wv8_d = nc.dram_tensor("wv8", [P, 3, 2, 384], F8, kind="ExternalInput").ap()
    wtb_d = nc.dram_tensor("wtb", [P, 6, 768], BF16, kind="ExternalInput").ap()
    wvb_d = nc.dram_tensor("wvb", [P, 6, 384], BF16, kind="ExternalInput").ap()
    wo_d = nc.dram_tensor("wo", [P, 3, 768], BF16, kind="ExternalInput").ap()
    bq_d = nc.dram_tensor("bq", [P, 3], F32, kind="ExternalInput").ap()
    out_d = nc.dram_tensor("out", [C, T], BF16, kind="ExternalOutput").ap()

    with tile.TileContext(nc) as tc, nc.allow_low_precision(
        reason="fp8 DoubleRow matmul pipeline with bf16 early region; "
        "f32 PSUM accumulation throughout"
    ):
        _emit(tc, xt8_d, xtb_d, wt8_d, wv8_d, wtb_d, wvb_d, wo_d, bq_d, out_d)
    nc.compile()
    return nc


def _emit(tc, xt8_d, xtb_d, wt8_d, wv8_d, wtb_d, wvb_d, wo_d, bq_d, out_d):
    nc = tc.nc
    rt = Router()
    out_r = out_d.rearrange("(ob p) t -> p ob t", p=P)  # [128, 6, 2048]

    with (
        tc.tile_pool(name="persist", bufs=1) as persist,
        tc.tile_pool(name="stage", bufs=3) as stage,
        tc.tile_pool(name="attp", bufs=3) as attp,
        tc.tile_pool(name="ps_k", bufs=2, space="PSUM") as ps_k,
        tc.tile_pool(name="ps_s", bufs=2, space="PSUM") as ps_s,
        tc.tile_pool(name="ps_y", bufs=2, space="PSUM") as ps_y,
    ):
        # ---------------- persistent SBUF ----------------
        xtb = persist.tile([P, 6, 512], BF16)
        wtb = persist.tile([P, 6, 768], BF16)
        bq = persist.tile([P, 3], F32)
        wvb = persist.tile([P, 6, 384], BF16)
        xt8 = persist.tile([P, 3, 2, T], F8)
        wt8 = persist.tile([P, 3, 2, 768], F8)
        wv8 = persist.tile([P, 3, 2, 384], F8)
        wo = persist.tile([P, 3, 768], BF16)
        for t_, d_ in [(xtb, xtb_d), (wtb, wtb_d), (bq, bq_d), (wvb, wvb_d),
                       (xt8, xt8_d), (wt8, wt8_d), (wv8, wv8_d), (wo, wo_d)]:
            nc.sync.dma_start(t_, d_)

        # fp8 pair-layout q/k for DoubleRow scores: partition halves 0:32 /
        # 32:64 hold even/odd heads of head-pair hp; pair (2j,2j+1) at row j.
        qp = persist.tile([64, 3, 2, T], F8)
        kp = persist.tile([64, 3, 2, T], F8)
        # bf16 q/k for the early region (tokens 0:512); partition = dim % 128
        qb = persist.tile([P, 3, 512], BF16)
        kb = persist.tile([P, 3, 512], BF16)
        # V token-major: [token-part, kbp, pair-slot, head, 64 v | 64 ones]
        v8 = persist.tile([P, 8, 2, HL, P], F8)
        vb = persist.tile([P, 4, HL, P], BF16)
        nc.gpsimd.memset(v8[:, :, :, :, HD:P], 1.0)
        nc.gpsimd.memset(vb[:, :, :, HD:P], 1.0)
        # y^T [128 j, jb, t] for the output projection
        ytp = persist.tile([P, 3, T], BF16)
        # staging for the q/k pair shuffle
        q8s = persist.tile([P, 3, 3 * 512], F8)   # tt1..3
        k8s = persist.tile([P, 3, 4 * 512], F8)   # tt0..3

        def conv(out, in_, scalar1=None, scalar2=None, eng=None):
            """Engine-routed convert out of PSUM."""
            cols = in_.free_size()
            e = eng or rt.pick(cols)
            if eng:
                rt.charge(eng, cols)
            if scalar1 is None:
                if e == "act":
                    nc.scalar.copy(out, in_)
                else:
                    nc.vector.tensor_copy(out, in_)
            elif scalar2 is None:
                if e == "act":
                    nc.scalar.mul(out, in_, scalar1)
                else:
                    nc.vector.tensor_scalar_mul(out, in_, scalar1)
            else:
                if e == "act":
                    nc.scalar.activation(out, in_, IDF, bias=scalar2,
                                         scale=scalar1)
                else:
                    nc.vector.tensor_scalar(
                        out, in_, scalar1, scalar2,
                        mybir.AluOpType.mult, mybir.AluOpType.add)

        # ---------------- QKV projections ----------------
        # bf16 q,k for tokens 0:512 (dims-major)
        for ob in range(6):
            pq = ps_k.tile([P, 512], F32, tag="mm")
            for cs in range(6):
                nc.tensor.matmul(pq, wtb[:, cs, ts(ob, P)], xtb[:, cs, :],
                                 start=(cs == 0), stop=(cs == 5))
            if ob < 3:
                conv(qb[:, ob, :], pq, 1.0, bq[:, ob : ob + 1])
            else:
                conv(kb[:, ob - 3, :], pq)
                conv(k8s[:, ob - 3, 0:512], pq)
        # bf16 V token-major (token blocks 0..3)
        for tb in range(4):
            pvt = ps_k.tile([P, 512], F32, tag="mm", name="pvt")
            pv = pvt[:, 0:384]
            for cs in range(6):
                nc.tensor.matmul(pv, xtb[:, cs, ts(tb, P)], wvb[:, cs, :],
                                 start=(cs == 0), stop=(cs == 5))
            pv3 = pv.rearrange("p (h d) -> p h d", d=HD)
            conv(vb[:, tb, :, 0:HD], pv3)
            conv(v8[:, tb // 2, tb % 2, :, 0:HD], pv3)
        # fp8 DoubleRow q,k for tokens 512:2048
        for ob in range(6):
            for tt in range(1, 4):
                pq = ps_k.tile([P, 512], F32, tag="mm")
                for g in range(3):
                    nc.tensor.matmul(
                        pq, wt8[:, g, :, ts(ob, P)], xt8[:, g, :, ts(tt, 512)],
                        start=(g == 0), stop=(g == 2), perf_mode=DR)
                if ob < 3:
                    conv(q8s[:, ob, ts(tt - 1, 512)], pq, 1.0 / WS,
                         bq[:, ob : ob + 1])
                else:
                    conv(k8s[:, ob - 3, ts(tt, 512)], pq, 1.0 / WS)

        # pair-shuffle DMAs: src [128 dims, 3, n] -> dst [64, 3, 2 slot, n]
        def shuffle_dma(dst, src, dst_c0, src_c0, n):
            for hb in range(2):
                for sl in range(2):
                    nc.sync.dma_start(
                        dst[32 * hb : 32 * hb + 32, :, sl,
                            dst_c0 : dst_c0 + n],
                        src[64 * hb + sl : 64 * hb + 64 : 2, :,
                            src_c0 : src_c0 + n],
                    )

        shuffle_dma(qp, q8s, 512, 0, 1536)
        shuffle_dma(kp, k8s, 0, 0, 2048)

        # fp8 DoubleRow V token-major (token blocks 4..15)
        for tb in range(4, 16):
            pvt = ps_k.tile([P, 512], F32, tag="mm", name="pvt")
            pv = pvt[:, 0:384]
            for g in range(3):
                nc.tensor.matmul(
                    pv, xt8[:, g, :, ts(tb, P)], wv8[:, g, :, :],
                    start=(g == 0), stop=(g == 2), perf_mode=DR)
            conv(v8[:, tb // 2, tb % 2, :, 0:HD],
                 pv.rearrange("p (h d) -> p h d", d=HD), 1.0 / WS)

        # ---------------- attention ----------------
        def emit_exp(att_t, sp, sl, c0, c1, fp8, eng=None):
            cols = c1 - c0 if sl is not None else 2 * (c1 - c0)
            e = eng or rt.pick(cols)
            if eng:
                rt.charge(eng, cols)
            src = sp[:, :, c0:c1] if sl is None else sp[:, sl, c0:c1]
            if sl is None:
                dsta = att_t[:, :, c0:c1]
            else:
                dsta = att_t[:, sl, c0:c1]
            if e == "act":
                nc.scalar.activation(dsta, src, EXPF, scale=SCALE)
            else:
                assert fp8
                dst8 = (att_t.bitcast(U8)[:, :, c0:c1] if sl is None
                        else att_t.bitcast(U8)[:, sl, c0:c1])
                nc.vector.tensor_scalar(
                    dst8, src, A8, B8,
                    mybir.AluOpType.mult, mybir.AluOpType.add)

        def emit_head(h, qt_cb=None):
            hb = h % 2
            hp = h // 2
            for qt in range(4):
                fp8 = qt > 0
                ya = ps_y.tile([P, 512], F32, tag="y", name=f"ya{h}_{qt}")
                nkbp = 2 * qt + 2
                dlo = 2 * qt
                att_of = {}
                pending = None
                ya_started = [False]

                def emit_pv(kbp):
                    if kbp < dlo:
                        regions = [(0, 512, "dr")]
                    elif kbp == dlo:
                        regions = [(128, 512, "dr"), (0, 128, "a")]
                    else:
                        regions = [(384, 512, "dr"), (256, 384, "a")]
                    att_t = att_of.pop(kbp)
                    last = kbp == nkbp - 1
                    for c0, c1, kind in regions:
                        # start=True pending-zeroes the WHOLE 2KB psum
                        # region: issue it exactly once per ya tile; later
                        # first-touch writes overwrite via cleared
                        # has_written bits.
                        st = not ya_started[0]
                        ya_started[0] = True
                        if fp8:
                            if kind == "dr":
                                nc.tensor.matmul(
                                    ya[:, c0:c1], v8[:, kbp, :, h, :],
                                    att_t[:, :, c0:c1],
                                    start=st, stop=last and kind == "dr",
                                    perf_mode=DR, skip_group_check=True)
                            else:
                                nc.tensor.matmul(
                                    ya[:, c0:c1], v8[:, kbp, 0, h, :],
                                    att_t[:, 0, c0:c1],
                                    start=st, stop=False,
                                    skip_group_check=True)
                        else:
                            for sl in range(2):
                                if kind == "a" and sl == 1:
                                    continue
                                nc.tensor.matmul(
                                    ya[:, c0:c1], vb[:, 2 * kbp + sl, h, :],
                                    att_t[:, sl, c0:c1],
                                    start=st and sl == 0,
                                    stop=last and kind == "dr" and sl == 1,
                                    skip_group_check=True)

                for kbp in range(nkbp):
                    # scores for kb = 2kbp, 2kbp+1 into one [128,2,512] slab
                    sp = ps_s.tile([P, 2, 512], F32, tag="s")
                    for sl in range(2):
                        kb_i = 2 * kbp + sl
                        if fp8:
                            nc.tensor.matmul(
                                sp[:, sl, :],
                                kp[32 * hb : 32 * hb + 32, hp, :, ts(kb_i, P)],
                                qp[32 * hb : 32 * hb + 32, hp, :, ts(qt, 512)],
                                start=True, stop=True, perf_mode=DR)
                        else:
                            nc.tensor.matmul(
                                sp[:, sl, :],
                                kb[64 * hb : 64 * hb + 64, hp, ts(kb_i, P)],
                                qb[64 * hb : 64 * hb + 64, hp, :],
                                start=True, stop=True)
                    att_t = attp.tile([P, 2, 512], F8 if fp8 else BF16,
                                      tag="att" if fp8 else "attb")
                    att_of[kbp] = att_t
                    eng = None if fp8 else "act"
                    if kbp <= dlo:
                        emit_exp(att_t, sp, None, 0, 512, fp8, eng)
                    else:
                        emit_exp(att_t, sp, 0, 256, 512, fp8, eng)
                        emit_exp(att_t, sp, 1, 384, 512, fp8, eng)
                    # diagonal triangle masks
                    for sl in range(2):
                        c1p = (2 * kbp + sl) * P - qt * 512
                        if 0 <= c1p < 512:
                            if fp8:
                                va = att_t.bitcast(I8)[:, sl, c1p : c1p + P]
                                fill = 0
                            else:
                                va = att_t[:, sl, c1p : c1p + P]
                                fill = 0.0
                            nc.gpsimd.affine_select(
                                out=va, in_=va,
                                compare_op=mybir.AluOpType.is_ge,
                                fill=fill, base=0, channel_multiplier=-1,
                                pattern=[[1, P]])
                    if pending is not None:
                        emit_pv(pending)
                    pending = kbp
                emit_pv(pending)
                # normalize: reciprocal of replicated denom rows, multiply
                rden = stage.tile([64, 512], F32, tag="rden")
                nc.vector.reciprocal(rden, ya[64:128, :])
                rt.charge("dve", 512)
                dst = (ytp[0:64, hp, ts(qt, 512)] if hb == 0
                       else attp.tile([64, 512], BF16, tag="yodd"))
                nc.vector.tensor_mul(out=dst, in0=ya[0:64, :], in1=rden)
                rt.charge("dve", 512)
                if hb == 1:
                    nc.sync.dma_start(ytp[64:128, hp, ts(qt, 512)], dst)
                if qt_cb is not None:
                    qt_cb(qt)

        # ---------------- output projection ----------------
        def emit_outproj(tt):
            for ob in range(6):
                po = ps_k.tile([P, 512], F32, tag="mm")
                for jb in range(3):
                    nc.tensor.matmul(
                        po, wo[:, jb, ts(ob, P)], ytp[:, jb, ts(tt, 512)],
                        start=(jb == 0), stop=(jb == 2))
                osb = stage.tile([P, 512], BF16, tag="osb")
                conv(osb, po)
                nc.sync.dma_start(out_r[:, ob, ts(tt, 512)], osb)

        for h in range(5):
            emit_head(h)
        emit_head(5, qt_cb=emit_outproj)


_NC_CACHE = None
LAST_RESULTS = None


def _get_nc():
    global _NC_CACHE
    if _NC_CACHE is None:
        _NC_CACHE = _build_bass()
    return _NC_CACHE


def kernel(x, W_attn, b_attn, W_o, b_o):
    global LAST_RESULTS
    x = np.asarray(x, np.float32)
    W_attn = np.asarray(W_attn, np.float32)
    b_attn = np.asarray(b_attn, np.float32)
    W_o = np.asarray(W_o, np.float32)
    b_o = np.asarray(b_o, np.float32)
    B = x.shape[0]

    # c-dim pair permutation: entry (p, g, i) holds original c = 256g+2p+i
    pp, gg, ii = np.meshgrid(np.arange(P), np.arange(3), np.arange(2),
                             indexing="ij")
    cidx = (256 * gg + 2 * pp + ii).reshape(-1)

    in_maps = []
    for core in range(8):
        b, hg = divmod(core, 2)
        sl = slice(hg * J, (hg + 1) * J)
        Wq = W_attn[sl]
        Wk = W_attn[C + hg * J : C + (hg + 1) * J]
        Wv = W_attn[2 * C + hg * J : 2 * C + (hg + 1) * J]
        Wqk = np.concatenate([Wq, Wk], axis=0)          # [768, 768]
        xb = x[b]                                       # [T, C]

        in_maps.append({
            "xt8": np.ascontiguousarray(
                xb.T[cidx].reshape(P, 3, 2, T)).astype(NPF8),
            "xtb": np.ascontiguousarray(
                xb[:512].T.reshape(6, P, 512).transpose(1, 0, 2)
            ).astype(NPBF16),
            "wt8": np.ascontiguousarray(
                (Wqk.T * WS)[cidx].reshape(P, 3, 2, 768)).astype(NPF8),
            "wv8": np.ascontiguousarray(
                (Wv.T * WS)[cidx].reshape(P, 3, 2, J)).astype(NPF8),
            "wtb": np.ascontiguousarray(
                Wqk.T.reshape(6, P, 768).transpose(1, 0, 2)).astype(NPBF16),
            "wvb": np.ascontiguousarray(
                Wv.T.reshape(6, P, J).transpose(1, 0, 2)).astype(NPBF16),
            "wo": np.ascontiguousarray(
                W_o[:, sl].T.reshape(3, P, C).transpose(1, 0, 2)
            ).astype(NPBF16),
            "bq": np.ascontiguousarray(
                b_attn[sl].reshape(3, P).transpose(1, 0)).astype(np.float32),
        })

    nc = _get_nc()
    LAST_RESULTS = bass_utils.run_bass_kernel_spmd(
        nc, in_maps, core_ids=list(range(8)),
        trace=bool(int(os.environ.get("KERNEL_TRACE", "0"))),
    )
    parts = [r["out"].astype(np.float32) for r in LAST_RESULTS.results]

    bo_eff = b_o + W_o @ b_attn[2 * C :]
    out = np.empty((B, T, C), np.float32)
    for b in range(B):
        out[b] = (parts[2 * b] + parts[2 * b + 1]).T + bo_eff
    return out
